# revision 1
# baseline (speedup 1.0000x reference)
"""Ewald reciprocal-space kernel for Trainium2 (8 NeuronCores, SPMD).

Math (per batch b):
    s        = cell_inv @ x          (fractional coords)
    theta    = 2*pi * (kvec . s)     (B, N, NK) phases
    S_re/S_im= sum_n q_n {cos,sin}(theta)          (structure factor)
    recip_n  = sum_k expfac_k (S_re cos + S_im sin)
    phi      = recip * BOHR/(pi*V) - q * 2*bewald*BOHR/sqrt(pi)
    returns (0.5*q*phi, phi)

Sharding: 8 cores = 2 batches x 4 k-shards (1024 k-vectors each). Each core
computes its full-N, shard-K contribution to recip with no collectives; host
sums the 4 shard partials per batch and applies the final affine.

Device pipeline per core (N=4096 as 32 chunks of 128 partitions):
  u = x . (Cinv^T k)  [= theta/2pi]   via fp32r matmul (contraction dim 3)
  rn = (u + M) - M            magic-number round-to-nearest (DVE tensor_scalar)
  -r = rn - u                 in [-1/2, 1/2]   (DVE scalar_tensor_tensor)
  -r_c = wrap(-r - 1/4)       in [-1/2, 1/2]   (DVE add_range_wrap custom op)
  sin(theta) = Sin(-2pi * -r), cos(theta) = Sin(-2pi * -r_c)  (ACT, fp16 out)
  S_re/S_im: PE matmuls contracting n with q as weights (psum accumulate)
  cs chunks DMA-transposed (xbar) into csT[k-slice partitions, n free]
  w = expfac * S  (small), transposed to a [128,16] column tile via DRAM bounce
  recip: PE matmuls contracting k-slices: sum_j wcol_j^T @ csT_j
"""

import math
from contextlib import ExitStack

import numpy as np

BOHR = 1.8897261258369282

B, N, NK = 2, 4096, 4096
NCORES = 8
KSH = NK // 4          # k-vectors per core
NCH = N // 128         # 32 n-chunks
CW = 2 * KSH           # cs chunk width: [cos | sin]
NSL = CW // 128        # 16 k-slices per chunk

_PROG = {}


def _build_program(reps: int = 1, stage: str = 'full'):
    import concourse.bass as bass
    import concourse.bacc as bacc
    import concourse.tile as tile
    import concourse.mybir as mybir

    F32 = mybir.dt.float32
    F32R = mybir.dt.float32r
    F16 = mybir.dt.float16
    MAGIC = 12582912.0          # 1.5 * 2**23: fp32 round-to-nearest-integer
    # two fp32 ulps below 2*pi so |scale * r| <= pi holds for r = +-1/2 exactly
    NEG2PI = -6.28318452835083
    ADD = mybir.AluOpType.add
    SUB = mybir.AluOpType.subtract

    nc = bacc.Bacc(trn_type="TRN2", target_bir_lowering=False, debug=False)

    coordsT_d = nc.dram_tensor("coordsT", [3, N], F32, kind="ExternalInput").ap()
    qT_d = nc.dram_tensor("qT", [128, NCH], F32, kind="ExternalInput").ap()
    cinv_d = nc.dram_tensor("cinv", [3, 3], F32, kind="ExternalInput").ap()
    kvecT_d = nc.dram_tensor("kvecT", [3, KSH], F32, kind="ExternalInput").ap()
    expfac_d = nc.dram_tensor("expfac", [1, KSH], F32, kind="ExternalInput").ap()
    recip_d = nc.dram_tensor("recip", [1, N], F32, kind="ExternalOutput").ap()
    wb_d = nc.dram_tensor("w_bounce", [1, CW], F16)

    rep_markers = []
    with tile.TileContext(nc) as tc, ExitStack() as ctx:
        const = ctx.enter_context(tc.tile_pool(name="const", bufs=1))
        pu = ctx.enter_context(tc.tile_pool(name="pu", bufs=2, space="PSUM"))
        pacc = ctx.enter_context(tc.tile_pool(name="pacc", bufs=1, space="PSUM"))
        wk_rn = ctx.enter_context(tc.tile_pool(name="wk_rn", bufs=2))
        wk_mr = ctx.enter_context(tc.tile_pool(name="wk_mr", bufs=3))
        wk_cs = ctx.enter_context(tc.tile_pool(name="wk_cs", bufs=4))
        wk_out = ctx.enter_context(tc.tile_pool(name="wk_out", bufs=2))

        # ---- load inputs ----
        kvt = wk_cs.tile([3, KSH], F32R, tag="cs", name="kvt")
        nc.sync.dma_start(out=kvt[:, :], in_=kvecT_d.bitcast(F32R))
        cinv_t = const.tile([3, 3], F32R)
        nc.sync.dma_start(out=cinv_t[:, :], in_=cinv_d.bitcast(F32R))
        cts = const.tile([3, N], F32R)
        nc.sync.dma_start(out=cts[:, 0:1024], in_=coordsT_d[:, 0:1024].bitcast(F32R))
        qt = const.tile([128, NCH], F32)
        nc.sync.dma_start(out=qt[:, :], in_=qT_d)
        for h in range(1024, N, 1024):
            nc.sync.dma_start(
                out=cts[:, h : h + 1024],
                in_=coordsT_d[:, h : h + 1024].bitcast(F32R),
            )
        ef_a = wk_out.tile([1, 512], F32, tag="rr", name="ef_a")
        nc.sync.dma_start(out=ef_a[:, :], in_=expfac_d[:, 0:512])
        ef_b = wk_out.tile([1, 512], F32, tag="rr", name="ef_b")
        nc.sync.dma_start(out=ef_b[:, :], in_=expfac_d[:, 512:1024])

        qt16 = const.tile([128, NCH], F16)
        nc.scalar.copy(qt16[:, :], qt[:, :])

        # persistent across reps: the transposed trig store
        csT = const.tile([128, NSL, N], F16)   # [k-in-slice][slice j][n]
        for _rep in range(reps):
            if _rep > 0:
                tc.strict_bb_all_engine_barrier()
            # ---- kmodT[j, k] = sum_i cinv[i, j] * kvecT[i, k]  (= (Cinv^T k)^T) ----
            km_ps = pu.tile([128, KSH], F32, tag="u")
            for h in range(0, KSH, 512):
                nc.tensor.matmul(
                    km_ps[:3, h : h + 512], lhsT=cinv_t[:, :], rhs=kvt[:, h : h + 512],
                    start=True, stop=True,
                )
            kmod = const.tile([3, KSH], F32R)
            nc.vector.tensor_copy(kmod[:, 0:512], km_ps[:3, 0:512])
            nc.vector.tensor_copy(kmod[:, 512:1024], km_ps[:3, 512:1024])

            # chunks whose round-to-nearest runs on the scalar engine (balances
            # DVE vs ACT busy time; ~19/32 assisted)
            ASSIST = {int((i + 0.5) * NCH / 19) for i in range(19)}

            sab = [
                pacc.tile([1, 512], F32, tag=f"sab{j}", name=f"sab{j}") for j in range(4)
            ]

            # ---- pass 1: phases, trig, structure factors, transposes ----
            # Software-pipelined one chunk ahead: the (matmul -> round-to-nearest)
            # production for chunk t+1 is emitted before chunk t's Sin
            # activations, so the DVE's scalar_tensor_tensor never waits on a
            # busy scalar engine.
            def produce(t):
                u_ps = pu.tile([128, KSH], F32, tag="u", name=f"u{t}")
                for h in range(0, KSH, 512):
                    nc.tensor.matmul(
                        u_ps[:, h : h + 512],
                        lhsT=cts[:, 128 * t : 128 * (t + 1)],
                        rhs=kmod[:, h : h + 512],
                        start=True, stop=True,
                    )
                rn = wk_rn.tile([128, KSH], F32, tag="rn", name=f"rn{t}")
                if t in ASSIST and stage not in ('mm', 'rr'):
                    # scalar engine computes v = u + M; DVE then gets rn - u via
                    # (v - M) - u in one scalar_tensor_tensor
                    nc.scalar.activation(
                        rn[:, :], u_ps[:, :],
                        mybir.ActivationFunctionType.Copy, bias=MAGIC, scale=1.0,
                    )
                    s0 = MAGIC
                else:
                    nc.vector.tensor_scalar(
                        out=rn[:, :], in0=u_ps[:, :], scalar1=MAGIC, scalar2=MAGIC,
                        op0=ADD, op1=SUB,
                    )
                    s0 = 0.0
                return u_ps, rn, s0

            cur = produce(0)
            for t in range(NCH):
                u_ps, rn, s0 = cur
                if stage == 'mm':
                    dummy = wk_rn.tile([128, KSH], F32, tag="rn", name=f"d{t}")
                    nc.vector.tensor_copy(dummy[:, :], u_ps[:, :])
                    if t + 1 < NCH:
                        cur = produce(t + 1)
                    continue
                mm = wk_mr.tile([128, CW], F32)    # [-r | -r_c] halves
                nc.vector.scalar_tensor_tensor(
                    out=mm[:, 0:KSH], in0=rn[:, :], scalar=s0, in1=u_ps[:, :],
                    op0=ADD if s0 == 0.0 else SUB, op1=SUB,
                )
                nc.vector.add_range_wrap(
                    out=mm[:, KSH:CW], in_=mm[:, 0:KSH],
                    shift=-0.25, bound=0.5, period=1.0,
                )
                if t + 1 < NCH:
                    cur = produce(t + 1)
                if stage == 'rr':
                    continue
                # one Sin over both halves: cs = [sin(theta) | cos(theta)]
                cs = wk_cs.tile([128, CW], F16, tag="cs")
                nc.scalar.activation(
                    cs[:, :], mm[:, :],
                    mybir.ActivationFunctionType.Sin, bias=0.0, scale=NEG2PI,
                )
                for j in range(4):
                    nc.tensor.matmul(
                        sab[j][:, :],
                        lhsT=qt16[:, t : t + 1],
                        rhs=cs[:, 512 * j : 512 * (j + 1)],
                        start=(t == 0), stop=(t == NCH - 1),
                    )
                if stage == 'act':
                    continue
                # csT[p, j, 128t + n] = cs[n, 128j + p]
                nc.sync.dma_start_transpose(
                    out=csT[:, :, 128 * t : 128 * (t + 1)], in_=cs[:, :],
                )

            if stage != 'full':
                zz = wk_out.tile([1, 512], F32, tag="rr", name="zz")
                nc.vector.memset(zz[:, :], 0.0)
                for nf in range(0, N, 512):
                    nc.sync.dma_start(out=recip_d[:, nf : nf + 512], in_=zz[:, :])
                continue
            # ---- mid: w = expfac * S; transpose to column layout via DRAM ----
            w_row = const.tile([1, CW], F16, tag="w_row")
            for j in range(4):
                nc.vector.tensor_tensor(
                    out=w_row[:, 512 * j : 512 * (j + 1)],
                    in0=sab[j][:, :],
                    in1=(ef_a if j % 2 == 0 else ef_b)[:, :],
                    op=mybir.AluOpType.mult,
                )
            nc.sync.dma_start(out=wb_d.ap(), in_=w_row[:, :])
            wcolT = const.tile([128, NSL], F16, tag="wcolT")
            nc.sync.dma_start_transpose(
                out=wcolT[:, :], in_=wb_d.ap().rearrange("a (j p) -> (a j) p", p=128),
            )

            # ---- pass 2: recip = sum_j wcol_j^T @ csT_j  (contract k on PE) ----
            for nf in range(0, N, 512):
                pb = pu.tile([1, 512], F32, tag="u", name="pb")
                for j in range(NSL):
                    nc.tensor.matmul(
                        pb[:, :],
                        lhsT=wcolT[:, j : j + 1],
                        rhs=csT[:, j, nf : nf + 512],
                        start=(j == 0), stop=(j == NSL - 1),
                    )
                rr = wk_out.tile([1, 512], F32)
                nc.scalar.copy(rr[:, :], pb[:, :])
                nc.sync.dma_start(out=recip_d[:, nf : nf + 512], in_=rr[:, :])

    nc.compile()
    return nc


def _get_prog(reps: int = 1, stage: str = "full"):
    key = (reps, stage)
    if key not in _PROG:
        _PROG[key] = _build_program(reps, stage)
    return _PROG[key]


def _make_in_maps(coords, q, cell_inv, kvec, expfac):
    in_maps = []
    for c in range(NCORES):
        b, ks = divmod(c, NCORES // B)
        sl = slice(KSH * ks, KSH * (ks + 1))
        in_maps.append({
            "coordsT": np.ascontiguousarray(coords[b].T, dtype=np.float32),
            "qT": np.ascontiguousarray(q[b].reshape(NCH, 128).T, dtype=np.float32),
            "cinv": np.ascontiguousarray(cell_inv, dtype=np.float32),
            "kvecT": np.ascontiguousarray(kvec[sl].T, dtype=np.float32),
            "expfac": np.ascontiguousarray(expfac[sl][None, :], dtype=np.float32),
        })
    return in_maps


def _finalize(results, q, volume, bewald):
    recip = np.zeros((B, N), np.float32)
    for c in range(NCORES):
        b = c // (NCORES // B)
        recip[b] += results[c]["recip"][0]
    scale1 = np.float32(BOHR / (math.pi * float(volume[0])))
    scale2 = np.float32(2.0 * float(bewald[0]) * BOHR / math.sqrt(math.pi))
    phi = (recip * scale1 - q.astype(np.float32) * scale2).astype(np.float32)
    e = (np.float32(0.5) * q.astype(np.float32) * phi).astype(np.float32)
    return e, phi


def kernel(coords, q, cell_inv, kvec, expfac, volume, bewald):
    from concourse.bass_utils import run_bass_kernel_spmd

    nc = _get_prog()
    in_maps = _make_in_maps(coords, q, cell_inv, kvec, expfac)
    res = run_bass_kernel_spmd(nc, in_maps, list(range(NCORES))).results
    return _finalize(res, q, volume, bewald)



# revision 4
# speedup vs baseline: 1.3957x; 1.3957x over previous
"""Ewald reciprocal-space kernel for Trainium2 (8 NeuronCores, SPMD).

Math (per batch b):
    s        = cell_inv @ x          (fractional coords)
    theta    = 2*pi * (kvec . s)     (B, N, NK) phases
    S_re/S_im= sum_n q_n {cos,sin}(theta)          (structure factor)
    recip_n  = sum_k expfac_k (S_re cos + S_im sin)
    phi      = recip * BOHR/(pi*V) - q * 2*bewald*BOHR/sqrt(pi)
    returns (0.5*q*phi, phi)

Sharding: 8 cores = 2 batches x 4 k-shards (1024 k-vectors each). Each core
computes its full-N, shard-K contribution to recip with no collectives; host
sums the 4 shard partials per batch and applies the final affine.

Device pipeline per core (N=4096 as 32 chunks of 128 partitions):
  C = -r in PSUM via 4 accumulating matmuls: u, +M, -M, -u (M = 1.5*2^23
      magic; -u from host-negated coords; each accumulate rounds in fp32 so
      C = round(u) - u exactly, |C| <= 1/2).
  sin half: ACT Sin reads C from PSUM directly (scale -2pi), fp16 out.
  cos half: DVE add_range_wrap(C - 1/4) -> fp16 phases, Sin per 4-chunk batch.
  S: flipped matmuls (lhsT = 128-col cs slabs, rhs = q chunk -> [128,1] out,
     PE cost ~ output free size) -> per-chunk psum, DVE-accumulated in SBUF.
  cs chunks DMA-transposed (xbar) into csT - SP queue only (ACT-queue
     transposes corrupt data on real HW).
  recip: flipped matmuls, 16-deep psum chains per n-chunk -> [128, 32].
"""

import math
from contextlib import ExitStack

import numpy as np

BOHR = 1.8897261258369282

B, N, NK = 2, 4096, 4096
NCORES = 8
KSH = NK // 4          # k-vectors per core
NCH = N // 128         # 32 n-chunks
NSL = 2 * KSH // 128   # 16 slices: 0-7 sin(k=128s+p), 8-15 cos
QB = 2                 # chunks per cos-Sin batch

_PROG = {}


def _build_program():
    import concourse.bass as bass
    import concourse.bacc as bacc
    import concourse.tile as tile
    import concourse.mybir as mybir

    F32 = mybir.dt.float32
    F32R = mybir.dt.float32r
    F16 = mybir.dt.float16
    MAGIC = 12582912.0          # 1.5 * 2**23: fp32 round-to-nearest-integer
    # two fp32 ulps below 2*pi so |scale * r| <= pi holds for r = +-1/2 exactly
    NEG2PI = -6.28318452835083

    nc = bacc.Bacc(trn_type="TRN2", target_bir_lowering=False, debug=False)

    coordsT_d = nc.dram_tensor("coordsT", [3, N], F32, kind="ExternalInput").ap()
    coordsN_d = nc.dram_tensor("coordsN", [3, N], F32, kind="ExternalInput").ap()
    qT_d = nc.dram_tensor("qT", [128, NCH], F32, kind="ExternalInput").ap()
    kmT_d = nc.dram_tensor("kmT", [3, KSH], F32, kind="ExternalInput").ap()
    ef2_d = nc.dram_tensor("ef2", [128, NSL], F32, kind="ExternalInput").ap()
    recp_d = nc.dram_tensor("recp", [128, NCH], F32, kind="ExternalOutput").ap()

    with tile.TileContext(nc) as tc, ExitStack() as ctx:
        const = ctx.enter_context(tc.tile_pool(name="const", bufs=1))
        pu = ctx.enter_context(tc.tile_pool(name="pu", bufs=2, space="PSUM"))
        pacc = ctx.enter_context(tc.tile_pool(name="pacc", bufs=1, space="PSUM"))
        wk = ctx.enter_context(tc.tile_pool(name="wk", bufs=2))

        # ---- load inputs (ACT queue for loads is fine; transposes are not) ----
        cts = const.tile([3, N], F32R)
        nc.sync.dma_start(out=cts[:, :], in_=coordsT_d.bitcast(F32R))
        ctsn = const.tile([3, N], F32R)
        nc.scalar.dma_start(out=ctsn[:, :], in_=coordsN_d.bitcast(F32R))
        km = const.tile([3, KSH], F32R)
        nc.scalar.dma_start(out=km[:, :], in_=kmT_d.bitcast(F32R))
        qt = const.tile([128, NCH], F32)
        nc.sync.dma_start(out=qt[:, :], in_=qT_d)
        ef = const.tile([128, NSL], F32)
        nc.scalar.dma_start(out=ef[:, :], in_=ef2_d)

        qt16 = const.tile([128, NCH], F16)
        nc.vector.tensor_copy(qt16[:, :], qt[:, :])
        ones = const.tile([1, 128], F32R)
        nc.vector.memset(ones.bitcast(F32)[:, :], 1.0)
        mrow = const.tile([1, 512], F32R)
        nc.vector.memset(mrow.bitcast(F32)[:, :], MAGIC)
        mrow_n = const.tile([1, 512], F32R)
        nc.vector.memset(mrow_n.bitcast(F32)[:, :], -MAGIC)
        s_run = const.tile([128, NSL], F32)
        nc.vector.memset(s_run[:, :], 0.0)

        csT = const.tile([128, NSL, N], F16)

        def emit_chunk_front(t, ci, mmc, csb):
            """C psum (4-pass magic), wrap (cos phases), Sin (sin half)."""
            C = pu.tile([128, KSH], F32, tag="C", name=f"C{t}")
            for off in range(0, KSH, 512):
                co = C[:, off:off + 512]
                nc.tensor.matmul(co, lhsT=cts[:, 128 * t:128 * (t + 1)],
                                 rhs=km[:, off:off + 512], start=True, stop=False)
                nc.tensor.matmul(co, lhsT=ones[:, :], rhs=mrow[:, :],
                                 start=False, stop=False)
                nc.tensor.matmul(co, lhsT=ones[:, :], rhs=mrow_n[:, :],
                                 start=False, stop=False)
                nc.tensor.matmul(co, lhsT=ctsn[:, 128 * t:128 * (t + 1)],
                                 rhs=km[:, off:off + 512], start=False, stop=True)
            nc.vector.add_range_wrap(out=mmc[:, ci, :], in_=C[:, :],
                                     shift=-0.25, bound=0.5, period=1.0)
            nc.scalar.activation(csb[:, ci, 0:KSH], C[:, :],
                                 mybir.ActivationFunctionType.Sin,
                                 bias=0.0, scale=NEG2PI)

        def emit_chunk_back(t, ci, csb):
            """S matmuls + transpose for a chunk whose cs is complete."""
            cs = csb[:, ci, :]
            s_ch = pu.tile([128, NSL], F32, tag="S", name=f"sch{t}")
            for s in range(NSL):
                nc.tensor.matmul(s_ch[:, s:s + 1],
                                 lhsT=cs[:, 128 * s:128 * (s + 1)],
                                 rhs=qt16[:, t:t + 1], start=True, stop=True)
            nc.vector.tensor_tensor(out=s_run[:, :], in0=s_run[:, :],
                                    in1=s_ch[:, :], op=mybir.AluOpType.add)
            nc.sync.dma_start_transpose(
                out=csT[:, :, 128 * t:128 * (t + 1)], in_=cs)

        NB = NCH // QB
        prev = None           # (tb, csb) of the previous batch
        for tb in range(0, NCH, QB):
            mmc = wk.tile([128, QB, KSH], F16, tag="mmc", name=f"mmc{tb}")
            csb = wk.tile([128, QB, 2 * KSH], F16, tag="cs", name=f"cs{tb}")
            for ci in range(QB):
                emit_chunk_front(tb + ci, ci, mmc, csb)
                # previous batch's S/transpose interleaved for pipelining
                if prev is not None:
                    emit_chunk_back(prev[0] + ci, ci, prev[1])
            nc.scalar.activation(csb[:, :, KSH:2 * KSH], mmc[:, :, :],
                                 mybir.ActivationFunctionType.Sin,
                                 bias=0.0, scale=NEG2PI)
            prev = (tb, csb)
        for ci in range(QB):
            emit_chunk_back(prev[0] + ci, ci, prev[1])

        # ---- w = expfac * S (fp16 columns) ----
        w = const.tile([128, NSL], F16)
        nc.vector.tensor_tensor(out=w[:, :], in0=s_run[:, :], in1=ef[:, :],
                                op=mybir.AluOpType.mult)

        # ---- pass 2: recip[128c+p] via 16-deep flipped-matmul psum chains ----
        r_acc = pacc.tile([128, NCH], F32, name="r_acc")
        outsb = const.tile([128, NCH], F32)
        for c in range(NCH):
            for s in range(NSL):
                nc.tensor.matmul(r_acc[:, c:c + 1],
                                 lhsT=csT[:, s, 128 * c:128 * (c + 1)],
                                 rhs=w[:, s:s + 1],
                                 start=(s == 0), stop=(s == NSL - 1))
        nc.vector.tensor_copy(outsb[:, :], r_acc[:, :])
        nc.sync.dma_start(out=recp_d, in_=outsb[:, :])

    nc.compile()
    return nc


def _get_prog():
    if "prog" not in _PROG:
        _PROG["prog"] = _build_program()
    return _PROG["prog"]


def _make_in_maps(coords, q, cell_inv, kvec, expfac):
    in_maps = []
    for c in range(NCORES):
        b, ks = divmod(c, NCORES // B)
        sl = slice(KSH * ks, KSH * (ks + 1))
        ct = np.ascontiguousarray(coords[b].T, dtype=np.float32)
        ef = np.ascontiguousarray(expfac[sl], dtype=np.float32)
        ef8 = ef.reshape(NSL // 2, 128).T          # [128, 8]
        in_maps.append({
            "coordsT": ct,
            "coordsN": np.ascontiguousarray(-ct),
            "qT": np.ascontiguousarray(q[b].reshape(NCH, 128).T, dtype=np.float32),
            "kmT": np.ascontiguousarray(
                (kvec[sl].astype(np.float32) @ cell_inv.astype(np.float32)).T),
            "ef2": np.ascontiguousarray(np.concatenate([ef8, ef8], axis=1)),
        })
    return in_maps


def _finalize(results, q, volume, bewald):
    recip = np.zeros((B, N), np.float32)
    for c in range(NCORES):
        b = c // (NCORES // B)
        recip[b] += results[c]["recp"].T.reshape(-1)
    scale1 = np.float32(BOHR / (math.pi * float(volume[0])))
    scale2 = np.float32(2.0 * float(bewald[0]) * BOHR / math.sqrt(math.pi))
    phi = (recip * scale1 - q.astype(np.float32) * scale2).astype(np.float32)
    e = (np.float32(0.5) * q.astype(np.float32) * phi).astype(np.float32)
    return e, phi


def kernel(coords, q, cell_inv, kvec, expfac, volume, bewald):
    from concourse.bass_utils import run_bass_kernel_spmd

    nc = _get_prog()
    in_maps = _make_in_maps(coords, q, cell_inv, kvec, expfac)
    res = run_bass_kernel_spmd(nc, in_maps, list(range(NCORES))).results
    return _finalize(res, q, volume, bewald)


# revision 7
# speedup vs baseline: 1.5059x; 1.0790x over previous
"""Ewald reciprocal-space kernel for Trainium2 (8 NeuronCores, SPMD).

Math (per batch b):
    s        = cell_inv @ x          (fractional coords)
    theta    = 2*pi * (kvec . s)     (B, N, NK) phases
    S_re/S_im= sum_n q_n {cos,sin}(theta)          (structure factor)
    recip_n  = sum_k expfac_k (S_re cos + S_im sin)
    phi      = recip * BOHR/(pi*V) - q * 2*bewald*BOHR/sqrt(pi)
    returns (0.5*q*phi, phi)

Sharding: 8 cores = 2 batches x 4 k-shards (1024 k-vectors each). Each core
computes its full-N, shard-K contribution to recip with no collectives; host
sums the 4 shard partials per batch and applies the final affine.

Device pipeline per core (N=4096 as 32 chunks of 128 partitions):
  C = -r in PSUM via 4 accumulating matmuls: u, +M, -M, -u (M = 1.5*2^23
      magic; -u from host-negated coords; each accumulate rounds in fp32 so
      C = round(u) - u exactly, |C| <= 1/2).
  sin half: ACT Sin reads C from PSUM directly (scale -2pi), fp16 out.
  cos half: DVE add_range_wrap(C - 1/4) -> fp16 phases, Sin per 4-chunk batch.
  S: flipped matmuls (lhsT = 128-col cs slabs, rhs = q chunk -> [128,1] out,
     PE cost ~ output free size) -> per-chunk psum, DVE-accumulated in SBUF.
  cs chunks DMA-transposed (xbar) into csT - SP queue only (ACT-queue
     transposes corrupt data on real HW).
  recip: flipped matmuls, 16-deep psum chains per n-chunk -> [128, 32].
"""

import math
from contextlib import ExitStack

import numpy as np

BOHR = 1.8897261258369282

B, N, NK = 2, 4096, 4096
NCORES = 8
KSH = NK // 4          # k-vectors per core
NCH = N // 128         # 32 n-chunks
NSL = 2 * KSH // 128   # 16 slices: 0-7 sin(k=128s+p), 8-15 cos
QB = 2                 # chunks per cos-Sin batch

_PROG = {}


def _build_program():
    import concourse.bass as bass
    import concourse.bacc as bacc
    import concourse.tile as tile
    import concourse.mybir as mybir

    F32 = mybir.dt.float32
    F32R = mybir.dt.float32r
    F16 = mybir.dt.float16
    MAGIC = 12582912.0          # 1.5 * 2**23: fp32 round-to-nearest-integer
    # two fp32 ulps below 2*pi so |scale * r| <= pi holds for r = +-1/2 exactly
    NEG2PI = -6.28318452835083

    nc = bacc.Bacc(trn_type="TRN2", target_bir_lowering=False, debug=False)

    coordsT_d = nc.dram_tensor("coordsT", [3, N], F32, kind="ExternalInput").ap()
    coordsN_d = nc.dram_tensor("coordsN", [3, N], F32, kind="ExternalInput").ap()
    qT_d = nc.dram_tensor("qT", [128, NCH], F32, kind="ExternalInput").ap()
    kmT_d = nc.dram_tensor("kmT", [3, KSH], F32, kind="ExternalInput").ap()
    ef2_d = nc.dram_tensor("ef2", [128, NSL], F32, kind="ExternalInput").ap()
    recp_d = nc.dram_tensor("recp", [128, NCH], F32, kind="ExternalOutput").ap()

    with tile.TileContext(nc) as tc, ExitStack() as ctx:
        const = ctx.enter_context(tc.tile_pool(name="const", bufs=1))
        pu = ctx.enter_context(tc.tile_pool(name="pu", bufs=3, space="PSUM"))
        psm = ctx.enter_context(tc.tile_pool(name="psm", bufs=1, space="PSUM"))
        pacc = ctx.enter_context(tc.tile_pool(name="pacc", bufs=1, space="PSUM"))
        wk = ctx.enter_context(tc.tile_pool(name="wk", bufs=2))

        # ---- load inputs on SP, PE-critical tensors first ----
        cts = const.tile([3, N], F32R)
        nc.sync.dma_start(out=cts[:, :], in_=coordsT_d.bitcast(F32R))
        km = const.tile([3, KSH], F32R)
        nc.sync.dma_start(out=km[:, :], in_=kmT_d.bitcast(F32R))
        ctsn = const.tile([3, N], F32R)
        nc.sync.dma_start(out=ctsn[:, :], in_=coordsN_d.bitcast(F32R))
        qt = const.tile([128, NCH], F32)
        nc.sync.dma_start(out=qt[:, :], in_=qT_d)
        ef = const.tile([128, NSL], F32)
        nc.sync.dma_start(out=ef[:, :], in_=ef2_d)

        qt16 = const.tile([128, NCH], F16)
        nc.vector.tensor_copy(qt16[:, :], qt[:, :])
        ones = const.tile([1, 128], F32R)
        nc.vector.memset(ones.bitcast(F32)[:, :], 1.0)
        mrow = const.tile([1, 512], F32R)
        nc.vector.memset(mrow.bitcast(F32)[:, :], MAGIC)
        mrow_n = const.tile([1, 512], F32R)
        nc.vector.memset(mrow_n.bitcast(F32)[:, :], -MAGIC)
        s_run = const.tile([128, NSL], F32)
        nc.vector.memset(s_run[:, :], 0.0)

        csT = const.tile([128, NSL, N], F16)

        def emit_chunk_front(t, ci, mmc, csb):
            """C psum (4-pass magic), wrap (cos phases), Sin (sin half)."""
            C = pu.tile([128, KSH], F32, tag="C", name=f"C{t}")
            for off in range(0, KSH, 512):
                co = C[:, off:off + 512]
                nc.tensor.matmul(co, lhsT=cts[:, 128 * t:128 * (t + 1)],
                                 rhs=km[:, off:off + 512], start=True, stop=False)
                nc.tensor.matmul(co, lhsT=ones[:, :], rhs=mrow[:, :],
                                 start=False, stop=False)
                nc.tensor.matmul(co, lhsT=ones[:, :], rhs=mrow_n[:, :],
                                 start=False, stop=False)
                nc.tensor.matmul(co, lhsT=ctsn[:, 128 * t:128 * (t + 1)],
                                 rhs=km[:, off:off + 512], start=False, stop=True)
            nc.vector.add_range_wrap(out=mmc[:, ci, :], in_=C[:, :],
                                     shift=-0.25, bound=0.5, period=1.0)
            nc.scalar.activation(csb[:, ci, 0:KSH], C[:, :],
                                 mybir.ActivationFunctionType.Sin,
                                 bias=0.0, scale=NEG2PI)

        def emit_chunk_back(t, ci, csb):
            """S matmuls + transpose for a chunk whose cs is complete."""
            cs = csb[:, ci, :]
            s_ch = psm.tile([128, NSL], F32, tag="S", name=f"sch{t}")
            for s in range(NSL):
                nc.tensor.matmul(s_ch[:, s:s + 1],
                                 lhsT=cs[:, 128 * s:128 * (s + 1)],
                                 rhs=qt16[:, t:t + 1], start=True, stop=True)
            nc.vector.tensor_tensor(out=s_run[:, :], in0=s_run[:, :],
                                    in1=s_ch[:, :], op=mybir.AluOpType.add)
            nc.sync.dma_start_transpose(
                out=csT[:, :, 128 * t:128 * (t + 1)], in_=cs)

        NB = NCH // QB
        prev = None           # (tb, csb) of the previous batch
        for tb in range(0, NCH, QB):
            mmc = wk.tile([128, QB, KSH], F16, tag="mmc", name=f"mmc{tb}")
            csb = wk.tile([128, QB, 2 * KSH], F16, tag="cs", name=f"cs{tb}")
            for ci in range(QB):
                emit_chunk_front(tb + ci, ci, mmc, csb)
                # previous batch's S/transpose interleaved for pipelining
                if prev is not None:
                    emit_chunk_back(prev[0] + ci, ci, prev[1])
            nc.scalar.activation(csb[:, :, KSH:2 * KSH], mmc[:, :, :],
                                 mybir.ActivationFunctionType.Sin,
                                 bias=0.0, scale=NEG2PI)
            prev = (tb, csb)
        for ci in range(QB):
            emit_chunk_back(prev[0] + ci, ci, prev[1])

        # ---- w = expfac * S (fp16 columns) ----
        w = const.tile([128, NSL], F16)
        nc.vector.tensor_tensor(out=w[:, :], in0=s_run[:, :], in1=ef[:, :],
                                op=mybir.AluOpType.mult)

        # ---- pass 2: recip[128c+p] via 16-deep flipped-matmul psum chains ----
        r_acc = pacc.tile([128, NCH], F32, name="r_acc")
        outsb = const.tile([128, NCH], F32)
        for c in range(NCH):
            for s in range(NSL):
                nc.tensor.matmul(r_acc[:, c:c + 1],
                                 lhsT=csT[:, s, 128 * c:128 * (c + 1)],
                                 rhs=w[:, s:s + 1],
                                 start=(s == 0), stop=(s == NSL - 1))
        nc.vector.tensor_copy(outsb[:, :], r_acc[:, :])
        nc.sync.dma_start(out=recp_d, in_=outsb[:, :])

    nc.compile()
    return nc


def _get_prog():
    if "prog" not in _PROG:
        _PROG["prog"] = _build_program()
    return _PROG["prog"]


def _make_in_maps(coords, q, cell_inv, kvec, expfac):
    in_maps = []
    for c in range(NCORES):
        b, ks = divmod(c, NCORES // B)
        sl = slice(KSH * ks, KSH * (ks + 1))
        ct = np.ascontiguousarray(coords[b].T, dtype=np.float32)
        ef = np.ascontiguousarray(expfac[sl], dtype=np.float32)
        ef8 = ef.reshape(NSL // 2, 128).T          # [128, 8]
        in_maps.append({
            "coordsT": ct,
            "coordsN": np.ascontiguousarray(-ct),
            "qT": np.ascontiguousarray(q[b].reshape(NCH, 128).T, dtype=np.float32),
            "kmT": np.ascontiguousarray(
                (kvec[sl].astype(np.float32) @ cell_inv.astype(np.float32)).T),
            "ef2": np.ascontiguousarray(np.concatenate([ef8, ef8], axis=1)),
        })
    return in_maps


def _finalize(results, q, volume, bewald):
    recip = np.zeros((B, N), np.float32)
    for c in range(NCORES):
        b = c // (NCORES // B)
        recip[b] += results[c]["recp"].T.reshape(-1)
    scale1 = np.float32(BOHR / (math.pi * float(volume[0])))
    scale2 = np.float32(2.0 * float(bewald[0]) * BOHR / math.sqrt(math.pi))
    phi = (recip * scale1 - q.astype(np.float32) * scale2).astype(np.float32)
    e = (np.float32(0.5) * q.astype(np.float32) * phi).astype(np.float32)
    return e, phi


def kernel(coords, q, cell_inv, kvec, expfac, volume, bewald):
    from concourse.bass_utils import run_bass_kernel_spmd

    nc = _get_prog()
    in_maps = _make_in_maps(coords, q, cell_inv, kvec, expfac)
    res = run_bass_kernel_spmd(nc, in_maps, list(range(NCORES))).results
    return _finalize(res, q, volume, bewald)


# revision 9
# speedup vs baseline: 1.6319x; 1.0837x over previous
"""Ewald reciprocal-space kernel for Trainium2 (8 NeuronCores, SPMD).

Math (per batch b):
    s        = cell_inv @ x          (fractional coords)
    theta    = 2*pi * (kvec . s)     (B, N, NK) phases
    S_re/S_im= sum_n q_n {cos,sin}(theta)          (structure factor)
    recip_n  = sum_k expfac_k (S_re cos + S_im sin)
    phi      = recip * BOHR/(pi*V) - q * 2*bewald*BOHR/sqrt(pi)
    returns (0.5*q*phi, phi)

Sharding: 8 cores = 2 batches x 4 k-shards (1024 k-vectors each). Each core
computes its full-N, shard-K contribution to recip with no collectives; host
sums the 4 shard partials per batch and applies the final affine.

Device pipeline per core (N=4096 as 32 chunks of 128 partitions):
  C = -r in PSUM via 4 accumulating matmuls: u, +M, -M, -u (M = 1.5*2^23
      magic; -u from host-negated coords; each accumulate rounds in fp32 so
      C = round(u) - u exactly, |C| <= 1/2).
  sin half: ACT Sin reads C from PSUM directly (scale -2pi), fp16 out.
  cos half: DVE add_range_wrap(C - 1/4) -> fp16 phases, Sin per 4-chunk batch.
  S: flipped matmuls (lhsT = 128-col cs slabs, rhs = q chunk -> [128,1] out,
     PE cost ~ output free size) -> per-chunk psum, DVE-accumulated in SBUF.
  cs chunks DMA-transposed (xbar) into csT - SP queue only (ACT-queue
     transposes corrupt data on real HW).
  recip: flipped matmuls, 16-deep psum chains per n-chunk -> [128, 32].
"""

import math
from contextlib import ExitStack

import numpy as np

BOHR = 1.8897261258369282

B, N, NK = 2, 4096, 4096
NCORES = 8
KSH = NK // 4          # k-vectors per core
NCH = N // 128         # 32 n-chunks
NSL = 2 * KSH // 128   # 16 slices: 0-7 sin(k=128s+p), 8-15 cos
QB = 2                 # chunks per cos-Sin batch

_PROG = {}


def _build_program():
    import concourse.bass as bass
    import concourse.bacc as bacc
    import concourse.tile as tile
    import concourse.mybir as mybir

    F32 = mybir.dt.float32
    F32R = mybir.dt.float32r
    F16 = mybir.dt.float16
    MAGIC = 12582912.0          # 1.5 * 2**23: fp32 round-to-nearest-integer
    # two fp32 ulps below 2*pi so |scale * r| <= pi holds for r = +-1/2 exactly
    NEG2PI = -6.28318452835083

    nc = bacc.Bacc(trn_type="TRN2", target_bir_lowering=False, debug=False)

    coordsT_d = nc.dram_tensor("coordsT", [3, N], F32, kind="ExternalInput").ap()
    coordsN_d = nc.dram_tensor("coordsN", [3, N], F32, kind="ExternalInput").ap()
    qT_d = nc.dram_tensor("qT", [128, NCH], F32, kind="ExternalInput").ap()
    kmT_d = nc.dram_tensor("kmT", [3, KSH], F32, kind="ExternalInput").ap()
    ef2_d = nc.dram_tensor("ef2", [128, NSL], F32, kind="ExternalInput").ap()
    recp_d = nc.dram_tensor("recp", [128, NCH], F32, kind="ExternalOutput").ap()

    with tile.TileContext(nc) as tc, ExitStack() as ctx:
        const = ctx.enter_context(tc.tile_pool(name="const", bufs=1))
        pu = ctx.enter_context(tc.tile_pool(name="pu", bufs=3, space="PSUM"))
        psm = ctx.enter_context(tc.tile_pool(name="psm", bufs=1, space="PSUM"))
        pacc = ctx.enter_context(tc.tile_pool(name="pacc", bufs=1, space="PSUM"))
        wk = ctx.enter_context(tc.tile_pool(name="wk", bufs=2))

        # ---- loads: PE-critical (cts head, km) on SP; rest on ACT queue ----
        cts = const.tile([3, N], F32R)
        nc.sync.dma_start(out=cts[:, 0:1024], in_=coordsT_d[:, 0:1024].bitcast(F32R))
        km = const.tile([3, KSH], F32R)
        nc.sync.dma_start(out=km[:, :], in_=kmT_d.bitcast(F32R))
        nc.sync.dma_start(out=cts[:, 1024:N],
                          in_=coordsT_d[:, 1024:N].bitcast(F32R))
        ctsn = const.tile([3, N], F32R)
        nc.scalar.dma_start(out=ctsn[:, :], in_=coordsN_d.bitcast(F32R))
        qt = const.tile([128, NCH], F32)
        nc.scalar.dma_start(out=qt[:, :], in_=qT_d)
        ef = const.tile([128, NSL], F32)
        nc.scalar.dma_start(out=ef[:, :], in_=ef2_d)

        qt16 = const.tile([128, NCH], F16)
        nc.vector.tensor_copy(qt16[:, :], qt[:, :])
        ones = const.tile([1, 128], F32R)
        nc.vector.memset(ones.bitcast(F32)[:, :], 1.0)
        mrow = const.tile([1, 512], F32R)
        nc.vector.memset(mrow.bitcast(F32)[:, :], MAGIC)
        mrow_n = const.tile([1, 512], F32R)
        nc.vector.memset(mrow_n.bitcast(F32)[:, :], -MAGIC)
        s_run = const.tile([128, NSL], F32)
        nc.vector.memset(s_run[:, :], 0.0)

        csT = const.tile([128, NSL, N], F16)

        def emit_chunk_front(t, ci, mmc, csb):
            """C psum (4-pass magic), wrap (cos phases), Sin (sin half)."""
            C = pu.tile([128, KSH], F32, tag="C", name=f"C{t}")
            for off in range(0, KSH, 512):
                co = C[:, off:off + 512]
                nc.tensor.matmul(co, lhsT=cts[:, 128 * t:128 * (t + 1)],
                                 rhs=km[:, off:off + 512], start=True, stop=False)
                nc.tensor.matmul(co, lhsT=ones[:, :], rhs=mrow[:, :],
                                 start=False, stop=False)
                nc.tensor.matmul(co, lhsT=ones[:, :], rhs=mrow_n[:, :],
                                 start=False, stop=False)
                nc.tensor.matmul(co, lhsT=ctsn[:, 128 * t:128 * (t + 1)],
                                 rhs=km[:, off:off + 512], start=False, stop=True)
            nc.vector.add_range_wrap(out=mmc[:, ci, :], in_=C[:, :],
                                     shift=-0.25, bound=0.5, period=1.0)
            nc.scalar.activation(csb[:, ci, 0:KSH], C[:, :],
                                 mybir.ActivationFunctionType.Sin,
                                 bias=0.0, scale=NEG2PI)

        def emit_chunk_back(t, ci, csb):
            """S matmuls + transpose for a chunk whose cs is complete."""
            cs = csb[:, ci, :]
            s_ch = psm.tile([128, NSL], F32, tag="S", name=f"sch{t}")
            for s in range(NSL):
                nc.tensor.matmul(s_ch[:, s:s + 1],
                                 lhsT=cs[:, 128 * s:128 * (s + 1)],
                                 rhs=qt16[:, t:t + 1], start=True, stop=True)
            nc.vector.tensor_tensor(out=s_run[:, :], in0=s_run[:, :],
                                    in1=s_ch[:, :], op=mybir.AluOpType.add)
            nc.sync.dma_start_transpose(
                out=csT[:, :, 128 * t:128 * (t + 1)], in_=cs)

        NB = NCH // QB
        prev = None           # (tb, csb) of the previous batch
        for tb in range(0, NCH, QB):
            mmc = wk.tile([128, QB, KSH], F16, tag="mmc", name=f"mmc{tb}")
            csb = wk.tile([128, QB, 2 * KSH], F16, tag="cs", name=f"cs{tb}")
            for ci in range(QB):
                emit_chunk_front(tb + ci, ci, mmc, csb)
                # previous batch's S/transpose interleaved for pipelining
                if prev is not None:
                    emit_chunk_back(prev[0] + ci, ci, prev[1])
            nc.scalar.activation(csb[:, :, KSH:2 * KSH], mmc[:, :, :],
                                 mybir.ActivationFunctionType.Sin,
                                 bias=0.0, scale=NEG2PI)
            prev = (tb, csb)
        for ci in range(QB):
            emit_chunk_back(prev[0] + ci, ci, prev[1])

        # ---- w = expfac * S (fp16 columns) ----
        w = const.tile([128, NSL], F16)
        nc.vector.tensor_tensor(out=w[:, :], in0=s_run[:, :], in1=ef[:, :],
                                op=mybir.AluOpType.mult)

        # ---- pass 2: recip[128c+p] via 16-deep flipped-matmul psum chains ----
        r_acc = pacc.tile([128, NCH], F32, name="r_acc")
        outsb = const.tile([128, NCH], F32)
        for c in range(NCH):
            for s in range(NSL):
                nc.tensor.matmul(r_acc[:, c:c + 1],
                                 lhsT=csT[:, s, 128 * c:128 * (c + 1)],
                                 rhs=w[:, s:s + 1],
                                 start=(s == 0), stop=(s == NSL - 1))
        nc.vector.tensor_copy(outsb[:, :], r_acc[:, :])
        nc.scalar.dma_start(out=recp_d, in_=outsb[:, :])

    nc.compile()
    return nc


def _get_prog():
    if "prog" not in _PROG:
        _PROG["prog"] = _build_program()
    return _PROG["prog"]


def _make_in_maps(coords, q, cell_inv, kvec, expfac):
    in_maps = []
    for c in range(NCORES):
        b, ks = divmod(c, NCORES // B)
        sl = slice(KSH * ks, KSH * (ks + 1))
        ct = np.ascontiguousarray(coords[b].T, dtype=np.float32)
        ef = np.ascontiguousarray(expfac[sl], dtype=np.float32)
        ef8 = ef.reshape(NSL // 2, 128).T          # [128, 8]
        in_maps.append({
            "coordsT": ct,
            "coordsN": np.ascontiguousarray(-ct),
            "qT": np.ascontiguousarray(q[b].reshape(NCH, 128).T, dtype=np.float32),
            "kmT": np.ascontiguousarray(
                (kvec[sl].astype(np.float32) @ cell_inv.astype(np.float32)).T),
            "ef2": np.ascontiguousarray(np.concatenate([ef8, ef8], axis=1)),
        })
    return in_maps


def _finalize(results, q, volume, bewald):
    recip = np.zeros((B, N), np.float32)
    for c in range(NCORES):
        b = c // (NCORES // B)
        recip[b] += results[c]["recp"].T.reshape(-1)
    scale1 = np.float32(BOHR / (math.pi * float(volume[0])))
    scale2 = np.float32(2.0 * float(bewald[0]) * BOHR / math.sqrt(math.pi))
    phi = (recip * scale1 - q.astype(np.float32) * scale2).astype(np.float32)
    e = (np.float32(0.5) * q.astype(np.float32) * phi).astype(np.float32)
    return e, phi


def kernel(coords, q, cell_inv, kvec, expfac, volume, bewald):
    from concourse.bass_utils import run_bass_kernel_spmd

    nc = _get_prog()
    in_maps = _make_in_maps(coords, q, cell_inv, kvec, expfac)
    res = run_bass_kernel_spmd(nc, in_maps, list(range(NCORES))).results
    return _finalize(res, q, volume, bewald)
